# revision 13
# baseline (speedup 1.0000x reference)
"""DeepseekV2 MLA attention on 8 Trainium2 NeuronCores.

Sharding (uniform SPMD, no control divergence):
- A-projection, q-up-projection, final W_O: token-sharded (core c owns
  tokens [256c, 256c+256)).
- Attention (absorbed MLA over the compressed KV latent): head-sharded
  (core c owns heads {2c, 2c+1}).
- Collectives: AllGather of the kv latent (feature-major, bf16), then
  TWO AllToAlls of q^T (split by 128-token halves m0/m1 of each core)
  and TWO AllToAlls of normalized o^T (same split).

Pipelining via the m-split: attention processes the EVEN 128-token
query blocks (= the m0 half of every core) first, so it starts as soon
as q-a2a#1 lands while q-a2a#2 is still on the wire; the o outputs of
the even blocks are exchanged (o-a2a#1) during odd-block attention, and
W_O runs a lag pipeline of (column-group, m) jobs: five m0 jobs run
before the first m1 job, hiding o-a2a#2.

All matmuls run in bf16 with fp32 PSUM accumulation. RMSNorm weights
are folded into the adjacent weight matrices on the host. Softmax runs
unnormalized with the denominator from a ones-column appended to V.
w_qb is host-permuted so q-up output groups are [nope-cols | pe-cols]:
nope PSUM copies cast straight to bf16, only the pe slice stays fp32
for rope. Weight streams use host-repacked per-partition-contiguous
layouts; queue assignment prioritizes the kv-path inputs at cold start.
"""

import os
import sys

for _p in ("/opt/trn_rl_repo", "/root/.axon_site", "/root/.axon_site/_ro/trn_rl_repo",
           "/root/.axon_site/_ro/pypackages"):
    if os.path.isdir(_p) and _p not in sys.path:
        sys.path.insert(0, _p)

import numpy as np
import ml_dtypes

import concourse.bass as bass
import concourse.tile as tile
from concourse import bacc, mybir
from concourse.bass_utils import run_bass_kernel_spmd
from concourse.masks import make_identity

# Problem constants (hardcoded per contract)
T, HID, H = 2048, 5120, 16
DN, DR, DV = 128, 64, 128
QL, KVL = 1536, 512
EPS = 1e-6
THETA = 10000.0
SCALE = (DN + DR) ** -0.5

NCORES = 8
TLOC = T // NCORES          # 256 tokens per core
HLOC = H // NCORES          # 2 heads per core
MCH = TLOC // 128           # 2 token chunks of 128
KD = HID // 128             # 40 contraction chunks for A-proj
QKD = QL // 128             # 12 contraction chunks for q-up
LC = KVL // 128             # 4 latent chunks
NQB = T // 128              # 16 query/key blocks of 128
DQK = DN + DR               # 192
DVE_ = DV + 1               # 129: extra ones-column for softmax denominator
NGQ = H * DQK // 512        # 6 q-up output groups
NGN = H * DN // 512         # 4 nope groups (after wqb permutation)
NHT = HID // 512            # 10 W_O output groups
WO_LAG = 5                  # m0 jobs in flight before first m1 job

BF = mybir.dt.bfloat16
F32 = mybir.dt.float32

_NC_CACHE = None
_last_in_maps = None


def _rope_pair(nc, pool, x_pairs, cos, sin, out_pairs, shape):
    """Interleaved rope: out1 = x1*cos - x2*sin ; out2 = x2*cos + x1*sin."""
    x1, x2 = x_pairs[:, 0], x_pairs[:, 1]
    o1, o2 = out_pairs[:, 0], out_pairs[:, 1]
    tm1 = pool.tile([128] + shape, F32, tag="rope_tm1", name="rope_tm1")
    tm2 = pool.tile([128] + shape, F32, tag="rope_tm2", name="rope_tm2")
    tm3 = pool.tile([128] + shape, F32, tag="rope_tm3", name="rope_tm3")
    nc.vector.tensor_mul(tm1[:], x1, cos)
    nc.vector.tensor_mul(tm2[:], x2, sin)
    nc.vector.tensor_mul(tm3[:], x1, sin)
    nc.vector.tensor_sub(o1, tm1[:], tm2[:])
    nc.vector.tensor_mul(tm1[:], x2, cos)
    nc.vector.tensor_add(o2, tm1[:], tm3[:])


def build_nc():
    nc = bacc.Bacc("TRN2", target_bir_lowering=False, debug=False,
                   num_devices=NCORES)

    # host-repacked, per-partition-contiguous inputs
    hT = nc.dram_tensor("hT", [4, 128, KD // 4 * TLOC], BF, kind="ExternalInput")
    wfkv = nc.dram_tensor("wfkv", [8, 128, KD // 8 * 576], BF, kind="ExternalInput")
    wfq = nc.dram_tensor("wfq", [3, 4, 128, KD // 4 * 512], BF, kind="ExternalInput")
    wqb = nc.dram_tensor("wqb", [NGQ, 128, QKD * 512], BF, kind="ExternalInput")
    wo = nc.dram_tensor("wo", [NHT, 128, H * 512], BF, kind="ExternalInput")
    cs = nc.dram_tensor("cs", [TLOC, DR], F32, kind="ExternalInput")
    wkcT = nc.dram_tensor("wkcT", [HLOC, DN, KVL], BF, kind="ExternalInput")
    wvc = nc.dram_tensor("wvc", [KVL, HLOC * DV], BF, kind="ExternalInput")
    cmask = nc.dram_tensor("cmask", [128, HLOC, 128], BF, kind="ExternalInput")
    out = nc.dram_tensor("out", [TLOC, HID], F32, kind="ExternalOutput")

    RG = [list(range(NCORES))]
    KQ4 = KD // 4  # 10 k-chunks per hT quarter / wfq tile
    KQ8 = KD // 8  # 5 k-chunks per wfkv tile

    with tile.TileContext(nc) as tc:
        consts_cm = tc.tile_pool(name="consts", bufs=1)
        consts = consts_cm.__enter__()
        dram_cm = tc.tile_pool(name="dram", bufs=1, space="DRAM")
        dram = dram_cm.__enter__()
        ps_mm_cm = tc.tile_pool(name="ps_mm", bufs=4, space="PSUM")
        ps_mm = ps_mm_cm.__enter__()
        ps_tr_cm = tc.tile_pool(name="ps_tr", bufs=2, space="PSUM")
        ps_tr = ps_tr_cm.__enter__()
        kvattn_cm = tc.tile_pool(name="kvattn", bufs=1)
        kvattn = kvattn_cm.__enter__()
        wpool_cm = tc.tile_pool(name="wpool", bufs=3)
        wpool = wpool_cm.__enter__()

        ident = consts.tile([128, 128], BF, name="ident")
        make_identity(nc, ident[:])
        eps_sb = consts.tile([128, 1], F32, name="eps_sb")
        nc.vector.memset(eps_sb[:], float(EPS))
        cmask_sb = consts.tile([128, HLOC, 128], BF, name="cmask_sb")
        wkc_sb = consts.tile([128, HLOC, KVL], BF, name="wkc_sb")
        wvc_sb = consts.tile([128, LC, HLOC * DV], BF, name="wvc_sb")

        # collective DRAM tiles
        ag_in = dram.tile([KVL + DR, TLOC], BF, name="ag_in")
        ag_out = dram.tile([NCORES, KVL + DR, TLOC], BF, addr_space="Shared",
                           name="ag_out")
        a2aq_in = [dram.tile([NCORES, HLOC, DQK, 128], BF, name=f"a2aq_in{m}")
                   for m in range(2)]
        a2aq_out = [dram.tile([NCORES, HLOC, DQK, 128], BF, name=f"a2aq_out{m}")
                    for m in range(2)]
        a2ao_in = [dram.tile([NCORES, HLOC, DV, 128], BF, name=f"a2ao_in{m}")
                   for m in range(2)]
        a2ao_out = [dram.tile([NCORES, HLOC, DV, 128], BF, name=f"a2ao_out{m}")
                    for m in range(2)]

        # ---------------- Stages 1-3: token-sharded projections -----------
        early_cm = tc.tile_pool(name="early", bufs=1)
        early = early_cm.__enter__()
        tmp_cm = tc.tile_pool(name="tmp", bufs=1)
        tmp = tmp_cm.__enter__()
        hT_cm = tc.tile_pool(name="hTp", bufs=1)
        hTp = hT_cm.__enter__()

        # cold-start load priority: the kv A-proj needs all of hT + wfkv
        # before it can finish, so those stream first, spread over three
        # queues in consumption order; wfq/wqb strictly behind them.
        hT_sb = hTp.tile([128, KD, TLOC], BF, name="hT_sb")
        qkv_sb = early.tile([128, MCH, QL + KVL + DR], F32, name="qkv_sb")
        wkv_t = []

        def _load_wkv(j, eng, split=False):
            wt = wpool.tile([128, KQ8, 576], BF, tag="w", name="wkv_t")
            if split:
                # first tile in two pieces so the first matmul starts sooner
                eng.dma_start(out=wt[:, :1, :].rearrange("p k c -> p (k c)"),
                              in_=wfkv[j][:, :576])
                eng.dma_start(out=wt[:, 1:, :].rearrange("p k c -> p (k c)"),
                              in_=wfkv[j][:, 576:])
            else:
                eng.dma_start(out=wt[:].rearrange("p k c -> p (k c)"), in_=wfkv[j])
            wkv_t.append(wt)

        for j in range(8):
            if j % 2 == 0:
                _load_wkv(j, nc.sync, split=(j == 0))
            else:
                _load_wkv(j, nc.gpsimd)
        # first hT quarter in two pieces as well
        nc.scalar.dma_start(
            out=hT_sb[:, :2, :].rearrange("p k t -> p (k t)"),
            in_=hT[0][:, :2 * TLOC])
        nc.scalar.dma_start(
            out=hT_sb[:, 2:KQ4, :].rearrange("p k t -> p (k t)"),
            in_=hT[0][:, 2 * TLOC:])
        for i in range(1, 4):
            nc.scalar.dma_start(
                out=hT_sb[:, i * KQ4:(i + 1) * KQ4, :].rearrange("p k t -> p (k t)"),
                in_=hT[i])

        # small consts behind the critical stream (scalar after hT;
        # gpsimd after its wfkv tiles)
        cs_sb = early.tile([128, MCH, DR], F32, name="cs_sb")
        nc.scalar.dma_start(out=cs_sb[:],
                            in_=cs[:, :].rearrange("(m p) d -> p m d", p=128))
        nc.gpsimd.dma_start(out=cmask_sb[:], in_=cmask[:, :, :])
        nc.gpsimd.dma_start(out=wkc_sb[:],
                            in_=wkcT[:, :, :].rearrange("h d l -> d h l"))
        nc.gpsimd.dma_start(out=wvc_sb[:],
                            in_=wvc[:, :].rearrange("(c p) v -> p c v", p=128))
        crep = early.tile([128, MCH, H, DR], F32, name="crep")
        _cs_base = cs[:, :]
        for m in range(MCH):
            nc.scalar.dma_start(
                out=crep[:, m],
                in_=bass.AP(tensor=_cs_base.tensor, offset=m * 128 * DR,
                            ap=[[DR, 128], [0, H], [1, DR]]))

        # ---- kv columns of the A-projection (fat streamed weights) -------
        kvps = [ps_mm.tile([128, 512], F32, tag="mm", name="kvps") for _ in range(MCH)]
        peps = [ps_mm.tile([128, 512], F32, tag="mm", name="peps") for _ in range(MCH)]
        for k in range(KD):
            wt = wkv_t[k // KQ8][:, k % KQ8, :]
            for m in range(MCH):
                nc.tensor.matmul(kvps[m][:], hT_sb[:, k, m * 128:(m + 1) * 128],
                                 wt[:, :KVL], start=(k == 0), stop=(k == KD - 1))
                nc.tensor.matmul(peps[m][:, :DR], hT_sb[:, k, m * 128:(m + 1) * 128],
                                 wt[:, KVL:], start=(k == 0), stop=(k == KD - 1))
        for m in range(MCH):
            nc.vector.tensor_copy(qkv_sb[:, m, QL:QL + KVL], kvps[m][:])
            nc.scalar.copy(qkv_sb[:, m, QL + KVL:], peps[m][:, :DR])

        # ---------------- Stage 2: kv latent + rope + AllGather -----------
        kvlat_bf = early.tile([128, MCH, KVL], BF, name="kvlat_bf")
        kpe_bf = early.tile([128, MCH, DR], BF, name="kpe_bf")
        agin_sb = early.tile([128, LC, MCH, 128], BF, name="agin_sb")
        agpe_sb = early.tile([64, MCH, 128], BF, name="agpe_sb")

        for m in range(MCH):
            sq = tmp.tile([128, QL], F32, tag="sq", name="sq_kv")[:, :KVL]
            ssum = tmp.tile([128, 1], F32, tag="ssum_kv", name="ssum_kv")
            nc.scalar.activation(sq, qkv_sb[:, m, QL:QL + KVL],
                                 mybir.ActivationFunctionType.Square,
                                 accum_out=ssum[:])
            rstd = tmp.tile([128, 1], F32, tag="rstd_kv", name="rstd_kv")
            nc.scalar.activation(rstd[:], ssum[:],
                                 mybir.ActivationFunctionType.Sqrt,
                                 bias=eps_sb[:], scale=1.0 / KVL)
            rinv = tmp.tile([128, 1], F32, tag="rinv_kv", name="rinv_kv")
            nc.vector.reciprocal(rinv[:], rstd[:])
            nc.vector.tensor_scalar_mul(kvlat_bf[:, m], in0=qkv_sb[:, m, QL:QL + KVL],
                                        scalar1=rinv[:])
            kv_pairs = qkv_sb[:, m, QL + KVL:].rearrange("p (i two) -> p two i", two=2)
            out_pairs = kpe_bf[:, m].rearrange("p (i two) -> p two i", two=2)
            _rope_pair(nc, tmp, kv_pairs,
                       cs_sb[:, m, :DR // 2], cs_sb[:, m, DR // 2:],
                       out_pairs, [DR // 2])
            for lc in range(LC):
                pt = ps_tr.tile([128, 128], BF, tag="tr", name="pt_tr")
                nc.tensor.transpose(pt[:], kvlat_bf[:, m, lc * 128:(lc + 1) * 128],
                                    ident[:])
                nc.vector.tensor_copy(agin_sb[:, lc, m, :], pt[:])
            ptp = ps_tr.tile([64, 128], BF, tag="tr", name="ptp_tr")
            nc.tensor.transpose(ptp[:], kpe_bf[:, m], ident[:])
            nc.vector.tensor_copy(agpe_sb[:, m, :], ptp[:])

        nc.gpsimd.dma_start(
            out=ag_in[:KVL, :].rearrange("(c p) t -> p c t", p=128),
            in_=agin_sb[:].rearrange("p c m t -> p c (m t)"))
        nc.gpsimd.dma_start(
            out=ag_in[KVL:, :],
            in_=agpe_sb[:].rearrange("p m t -> p (m t)"))
        nc.gpsimd.collective_compute(
            "AllGather", mybir.AluOpType.bypass, replica_groups=RG,
            ins=[ag_in.opt()], outs=[ag_out.opt()])

        # gathered-K loads on the gpsimd queue, right behind the AllGather
        kT_sb = kvattn.tile([128, LC, T], BF, name="kT_sb")
        for lc in range(LC):
            nc.gpsimd.dma_start(
                out=kT_sb[:, lc].rearrange("p (s t) -> p s t", s=NCORES),
                in_=ag_out[:, lc * 128:(lc + 1) * 128, :].rearrange("s p t -> p s t"))
        kTpe_sb = kvattn.tile([64, T], BF, name="kTpe_sb")
        nc.gpsimd.dma_start(
            out=kTpe_sb[:].rearrange("p (s t) -> p s t", s=NCORES),
            in_=ag_out[:, KVL:, :].rearrange("s p t -> p s t"))

        # ---------------- Stage 3: q path ---------------------------------
        # q columns of the A-projection: 3 output groups of 512
        for g in range(3):
            wq_t = []
            for i in range(4):
                wt = wpool.tile([128, KQ4, 512], BF, tag="w", name="wq_t")
                eng = nc.sync if i % 2 == 0 else nc.scalar
                eng.dma_start(out=wt[:].rearrange("p k c -> p (k c)"), in_=wfq[g, i])
                wq_t.append(wt)
            qps = [ps_mm.tile([128, 512], F32, tag="mm", name="qps")
                   for _ in range(MCH)]
            for k in range(KD):
                wt = wq_t[k // KQ4][:, k % KQ4, :]
                for m in range(MCH):
                    nc.tensor.matmul(qps[m][:], hT_sb[:, k, m * 128:(m + 1) * 128],
                                     wt[:], start=(k == 0), stop=(k == KD - 1))
            for m in range(MCH):
                if g % 2 == 0:
                    nc.scalar.copy(qkv_sb[:, m, g * 512:(g + 1) * 512], qps[m][:])
                else:
                    nc.vector.tensor_copy(qkv_sb[:, m, g * 512:(g + 1) * 512],
                                          qps[m][:])

        hT_cm.__exit__(None, None, None)

        qan_bf = early.tile([128, MCH, QL], BF, name="qan_bf")
        for m in range(MCH):
            sq2 = tmp.tile([128, QL], F32, tag="sq", name="sq_q")
            ssum2 = tmp.tile([128, 1], F32, tag="ssum_q", name="ssum_q")
            nc.scalar.activation(sq2[:], qkv_sb[:, m, :QL],
                                 mybir.ActivationFunctionType.Square,
                                 accum_out=ssum2[:])
            rstd2 = tmp.tile([128, 1], F32, tag="rstd_q", name="rstd_q")
            nc.scalar.activation(rstd2[:], ssum2[:],
                                 mybir.ActivationFunctionType.Sqrt,
                                 bias=eps_sb[:], scale=1.0 / QL)
            rinv2 = tmp.tile([128, 1], F32, tag="rinv_q", name="rinv_q")
            nc.vector.reciprocal(rinv2[:], rstd2[:])
            nc.vector.tensor_scalar_mul(qan_bf[:, m], in0=qkv_sb[:, m, :QL],
                                        scalar1=rinv2[:])

        qanT_sb = early.tile([128, QKD, TLOC], BF, name="qanT_sb")
        for m in range(MCH):
            for kc in range(QKD):
                pt = ps_tr.tile([128, 128], BF, tag="tr", name="pt_tr")
                nc.tensor.transpose(pt[:], qan_bf[:, m, kc * 128:(kc + 1) * 128],
                                    ident[:])
                nc.vector.tensor_copy(qanT_sb[:, kc, m * 128:(m + 1) * 128], pt[:])

        # q-up with host-permuted wqb: groups 0..3 = nope cols (cast to
        # bf16 straight from PSUM), groups 4..5 = pe cols (fp32 for rope).
        # V-precompute matmuls are interleaved to fill wqb DMA stalls.
        v_sb = kvattn.tile([128, NQB, HLOC, DVE_], BF, name="v_sb")
        nc.vector.memset(v_sb[:, :, :, DV:], 1.0)
        q_bf = early.tile([128, MCH, H, DN], BF, name="q_bf")
        qpe_sb = early.tile([128, MCH, H, DR], F32, name="qpe_sb")
        qpe_bf = early.tile([128, MCH, H, DR], BF, name="qpe_bf")

        def _vpre(tcb):
            pv = ps_mm.tile([128, HLOC * DV], F32, tag="mm", name="pv_ps")
            for lc in range(LC):
                nc.tensor.matmul(pv[:], kT_sb[:, lc, tcb * 128:(tcb + 1) * 128],
                                 wvc_sb[:, lc, :], start=(lc == 0),
                                 stop=(lc == LC - 1))
            nc.vector.tensor_copy(
                v_sb[:, tcb, :, :DV],
                pv[:].rearrange("p (h v) -> p h v", h=HLOC))

        # wqb gets its own pool: sharing wpool's rotation would make the
        # first wqb load wait (WAR) until A-proj-q consumed its wfq tiles.
        # Group order: pe groups first so rope (which needs all pe columns)
        # overlaps the nope-group matmuls; nope transposes interleave per
        # group so staging/a2a fire right after the last q-up matmul.
        wqbp_cm = tc.tile_pool(name="wqbp", bufs=2)
        wqbp = wqbp_cm.__enter__()
        aq0_sb = [early.tile([128, H, 128], BF, name=f"aq0_sb{m}")
                  for m in range(MCH)]
        aq1_sb = [early.tile([64, H, 128], BF, name=f"aq1_sb{m}")
                  for m in range(MCH)]
        vpre_done = 0
        for gi, ng in enumerate([NGN, NGN + 1, 0, 1, 2, 3]):
            wqb_t = wqbp.tile([128, QKD, 512], BF, tag="wqb", name="wqb_t")
            eng = nc.sync if gi % 2 == 0 else nc.scalar
            eng.dma_start(out=wqb_t[:].rearrange("p k c -> p (k c)"),
                          in_=wqb[ng])
            psq = [ps_mm.tile([128, 512], F32, tag="mm", name="qup_ps")
                   for _ in range(MCH)]
            for kc in range(QKD):
                for m in range(MCH):
                    nc.tensor.matmul(psq[m][:], qanT_sb[:, kc, m * 128:(m + 1) * 128],
                                     wqb_t[:, kc, :], start=(kc == 0),
                                     stop=(kc == QKD - 1))
            for m in range(MCH):
                if ng < NGN:
                    nc.vector.tensor_copy(
                        q_bf[:, m, 4 * ng:4 * ng + 4, :]
                            .rearrange("p h d -> p (h d)"), psq[m][:])
                else:
                    gg = ng - NGN
                    nc.scalar.copy(
                        qpe_sb[:, m, 8 * gg:8 * gg + 8, :]
                            .rearrange("p h d -> p (h d)"), psq[m][:])
            if ng == NGN + 1:
                # both pe groups done: rope + pe transposes for both m
                for m in range(MCH):
                    q_pairs = qpe_sb[:, m].rearrange("p h (i two) -> p two h i",
                                                     two=2)
                    o_pairs = qpe_bf[:, m].rearrange("p h (i two) -> p two h i",
                                                     two=2)
                    _rope_pair(nc, tmp, q_pairs,
                               crep[:, m, :, :DR // 2], crep[:, m, :, DR // 2:],
                               o_pairs, [H, DR // 2])
                    for hp in range(H // 2):
                        pt1 = ps_tr.tile([128, 128], BF, tag="tr", name="pt1_tr")
                        nc.tensor.transpose(
                            pt1[:],
                            qpe_bf[:, m, 2 * hp:2 * hp + 2, :]
                                .rearrange("p h d -> p (h d)"), ident[:])
                        nc.vector.tensor_copy(aq1_sb[m][:, 2 * hp, :], pt1[:64, :])
                        nc.vector.tensor_copy(aq1_sb[m][:, 2 * hp + 1, :],
                                              pt1[64:, :])
            if ng < NGN:
                for m in range(MCH):
                    for h in range(4 * ng, 4 * ng + 4):
                        pt0 = ps_tr.tile([128, 128], BF, tag="tr", name="pt0_tr")
                        nc.tensor.transpose(pt0[:], q_bf[:, m, h, :], ident[:])
                        nc.vector.tensor_copy(aq0_sb[m][:, h, :], pt0[:])
            # fill DMA stalls with V-precompute (kT is resident by now)
            while vpre_done < (gi + 1) * NQB // NGQ:
                _vpre(vpre_done)
                vpre_done += 1
        wqbp_cm.__exit__(None, None, None)

        # staging + AllToAll per token-half m
        for m in range(MCH):
            _aqv = a2aq_in[m][:, :, :, :].rearrange("s hh d t -> (s hh) d t") \
                                         .rearrange("h d t -> d h t")
            nc.gpsimd.dma_start(out=_aqv[:DN],
                                in_=aq0_sb[m][:].rearrange("p h t -> p (h t)"))
            nc.gpsimd.dma_start(out=_aqv[DN:],
                                in_=aq1_sb[m][:].rearrange("p h t -> p (h t)"))
            nc.gpsimd.collective_compute(
                "AllToAll", mybir.AluOpType.bypass, replica_groups=RG,
                ins=[a2aq_in[m].opt()], outs=[a2aq_out[m].opt()])

        tmp_cm.__exit__(None, None, None)
        early_cm.__exit__(None, None, None)
        wpool_cm.__exit__(None, None, None)
        attn_cm = tc.tile_pool(name="attn", bufs=1)
        attn = attn_cm.__enter__()
        wopool_cm = tc.tile_pool(name="wopool", bufs=6)
        wopool = wopool_cm.__enter__()
        oT_cm = tc.tile_pool(name="oTp", bufs=1)
        oTp = oT_cm.__enter__()

        wo_t = []
        for ht in range(NHT):
            wo_t.append(wopool.tile([128, H, 512], BF, tag="wo", name="wo_t"))

        def _load_wo(ht):
            eng = nc.sync if ht % 2 == 0 else nc.scalar
            eng.dma_start(out=wo_t[ht][:].rearrange("p c v -> p (c v)"),
                          in_=wo[ht])

        # ------------- Stage 5: q^T gathers + w_kc absorption -------------
        # pidx layout: column-permuted block index. Global q-block b maps
        # to pidx = b//2 for even b (m0 half) and 8 + b//2 for odd b.
        qT_sb = attn.tile([128, HLOC, T], BF, name="qT_sb")
        qTpe_sb = attn.tile([64, HLOC, T], BF, name="qTpe_sb")
        qabsT_sb = attn.tile([128, LC, NQB, HLOC, 128], BF, name="qabsT_sb")

        def _qabs(m):
            for h in range(HLOC):
                nc.sync.dma_start(
                    out=qT_sb[:, h, m * 1024:(m + 1) * 1024]
                        .rearrange("p (s t) -> p s t", s=NCORES),
                    in_=a2aq_out[m][:, h, :DN, :].rearrange("s d t -> d s t"))
                nc.sync.dma_start(
                    out=qTpe_sb[:, h, m * 1024:(m + 1) * 1024]
                        .rearrange("p (s t) -> p s t", s=NCORES),
                    in_=a2aq_out[m][:, h, DN:, :].rearrange("s d t -> d s t"))
            for h in range(HLOC):
                for tq in range(2):
                    base = m * 2 + tq
                    for lc in range(LC):
                        pqa = ps_mm.tile([128, 512], F32, tag="mm", name="pqa_ps")
                        nc.tensor.matmul(
                            pqa[:], wkc_sb[:, h, lc * 128:(lc + 1) * 128],
                            qT_sb[:, h, base * 512:(base + 1) * 512],
                            start=True, stop=True)
                        nc.scalar.copy(
                            qabsT_sb[:, lc, base * 4:(base + 1) * 4, h, :],
                            pqa[:].rearrange("p (q t) -> p q t", q=4))

        _qabs(0)

        # ---------------- Stage 6: attention ------------------------------
        ps_o_cm = tc.tile_pool(name="ps_o", bufs=2, space="PSUM")
        ps_o = ps_o_cm.__enter__()
        pexp_cm = tc.tile_pool(name="pexp", bufs=3)
        pexp = pexp_cm.__enter__()
        onorm_cm = tc.tile_pool(name="onorm", bufs=3)
        onorm = onorm_cm.__enter__()
        ao_sb = attn.tile([128, HLOC, NQB, 128], BF, name="ao_sb")
        oT_sb = oTp.tile([128, H, MCH, 128], BF, name="oT_sb")

        def _attn_pair(j, parity):
            """One pair of q-blocks (b0, b1) = (4j+parity, 4j+2+parity)."""
            b0, b1 = 4 * j + parity, 4 * j + 2 + parity
            p0 = 8 * parity + 2 * j          # pidx of b0; b1 is p0+1
            pos = [ps_o.tile([128, HLOC, DVE_], F32, tag="po", name="po0"),
                   ps_o.tile([128, HLOC, DVE_], F32, tag="po", name="po1")]
            for kb in range(b0 + 1):
                # both query blocks of the pair attend to this key block
                psc = ps_mm.tile([128, 2, HLOC, 128], F32, tag="mm",
                                 name="psc2")
                for lc in range(LC):
                    nc.tensor.matmul(
                        psc[:], kT_sb[:, lc, kb * 128:(kb + 1) * 128],
                        qabsT_sb[:, lc, p0:p0 + 2, :, :],
                        start=(lc == 0), stop=False)
                nc.tensor.matmul(
                    psc[:], kTpe_sb[:, kb * 128:(kb + 1) * 128],
                    qTpe_sb[:, :, p0 * 128:(p0 + 2) * 128]
                        .rearrange("p hh (q t) -> p q hh t", q=2),
                    start=False, stop=True)
                p_bf = pexp.tile([128, 2, HLOC, 128], BF, tag="p_bf",
                                 name="p_bf")
                nc.scalar.activation(p_bf[:], psc[:],
                                     mybir.ActivationFunctionType.Exp,
                                     scale=float(SCALE))
                if kb == b0:
                    nc.vector.tensor_mul(p_bf[:, 0], p_bf[:, 0], cmask_sb[:])
                for qi in range(2):
                    for h in range(HLOC):
                        nc.tensor.matmul(
                            pos[qi][:, h, :], p_bf[:, qi, h, :],
                            v_sb[:, kb, h, :],
                            start=(kb == 0 and h == 0),
                            stop=(kb == (b0 if qi == 0 else b1)))
            for kb in range(b0 + 1, b1 + 1):
                # only the second block attends to these key blocks
                psc1 = ps_mm.tile([128, HLOC, 128], F32, tag="mm",
                                  name="psc1")
                for lc in range(LC):
                    nc.tensor.matmul(
                        psc1[:], kT_sb[:, lc, kb * 128:(kb + 1) * 128],
                        qabsT_sb[:, lc, p0 + 1, :, :],
                        start=(lc == 0), stop=False)
                nc.tensor.matmul(
                    psc1[:], kTpe_sb[:, kb * 128:(kb + 1) * 128],
                    qTpe_sb[:, :, (p0 + 1) * 128:(p0 + 2) * 128],
                    start=False, stop=True)
                p1 = pexp.tile([128, HLOC, 128], BF, tag="p_bf", name="p1")
                nc.scalar.activation(p1[:], psc1[:],
                                     mybir.ActivationFunctionType.Exp,
                                     scale=float(SCALE))
                if kb == b1:
                    nc.vector.tensor_mul(p1[:], p1[:], cmask_sb[:])
                for h in range(HLOC):
                    nc.tensor.matmul(
                        pos[1][:, h, :], p1[:, h, :],
                        v_sb[:, kb, h, :],
                        start=False, stop=(kb == b1))
            for qi in range(2):
                po = pos[qi]
                for h in range(HLOC):
                    rh = onorm.tile([128, 1], F32, tag="rh", name="rh")
                    nc.vector.reciprocal(rh[:], po[:, h, DV:DVE_])
                    ob = onorm.tile([128, DV], BF, tag="ob", name="ob")
                    nc.vector.tensor_scalar_mul(ob[:], in0=po[:, h, :DV],
                                                scalar1=rh[:])
                    pot = ps_tr.tile([128, 128], BF, tag="tr", name="pot_tr")
                    nc.tensor.transpose(pot[:], ob[:], ident[:])
                    nc.scalar.copy(ao_sb[:, h, p0 + qi, :], pot[:])
            # stage this pair's output (dests 2j, 2j+1; half `parity`)
            for hh in range(HLOC):
                nc.gpsimd.dma_start(
                    out=a2ao_in[parity][2 * j:2 * j + 2, hh, :, :]
                        .rearrange("q2 v t -> v q2 t"),
                    in_=ao_sb[:, hh, p0:p0 + 2, :])

        # even-block pairs; wo prefetch rides along
        for j in range(4):
            if j >= 1:
                _load_wo(j - 1)
            _attn_pair(j, 0)
        nc.gpsimd.collective_compute(
            "AllToAll", mybir.AluOpType.bypass, replica_groups=RG,
            ins=[a2ao_in[0].opt()], outs=[a2ao_out[0].opt()])
        nc.gpsimd.dma_start(
            out=oT_sb[:, :, 0, :],
            in_=a2ao_out[0][:, :, :, :].rearrange("s hh v t -> v (s hh) t"))

        _qabs(1)

        # odd-block pairs
        for j in range(4):
            if j < 2:
                _load_wo(j + 3)
            _attn_pair(j, 1)
        nc.gpsimd.collective_compute(
            "AllToAll", mybir.AluOpType.bypass, replica_groups=RG,
            ins=[a2ao_in[1].opt()], outs=[a2ao_out[1].opt()])
        nc.gpsimd.dma_start(
            out=oT_sb[:, :, 1, :],
            in_=a2ao_out[1][:, :, :, :].rearrange("s hh v t -> v (s hh) t"))

        # ---------------- Stage 7: W_O lag pipeline -----------------------
        # job order: m0 jobs lead by WO_LAG so the second o-a2a and its
        # gather hide under real work; each wo tile serves its m0 job and
        # its m1 job WO_LAG slots later (wopool bufs=6 covers the span).
        jobs = []
        nm0 = nm1 = 0
        for s in range(2 * NHT):
            if nm0 < NHT and (nm0 < WO_LAG or nm0 - nm1 < WO_LAG or nm1 >= NHT):
                jobs.append((nm0, 0))
                nm0 += 1
            else:
                jobs.append((nm1, 1))
                nm1 += 1

        outp_cm = tc.tile_pool(name="outp", bufs=4)
        outp = outp_cm.__enter__()
        store_engs = [nc.sync, nc.scalar, nc.gpsimd]
        nwo_loaded = 5
        for si, (ht, m) in enumerate(jobs):
            if m == 0 and nwo_loaded < NHT:
                _load_wo(nwo_loaded)
                nwo_loaded += 1
            pso = ps_mm.tile([128, 512], F32, tag="mm", name="wo_ps")
            for c in range(H):
                nc.tensor.matmul(pso[:], oT_sb[:, c, m, :],
                                 wo_t[ht][:, c, :], start=(c == 0),
                                 stop=(c == H - 1))
            ot = outp.tile([128, 512], F32, tag="ot", name="ot")
            if si % 2 == 0:
                nc.scalar.copy(ot[:], pso[:])
            else:
                nc.vector.tensor_copy(ot[:], pso[:])
            store_engs[si % 3].dma_start(
                out=out[:, :].rearrange("(m p) d -> p m d", p=128)[
                    :, m, ht * 512:(ht + 1) * 512],
                in_=ot[:])

        for p in (outp_cm, onorm_cm, pexp_cm, ps_o_cm, oT_cm, wopool_cm,
                  attn_cm, kvattn_cm, ps_tr_cm, ps_mm_cm, dram_cm, consts_cm):
            p.__exit__(None, None, None)

    nc.finalize()
    return nc


def _to_bf16(a):
    return np.asarray(a, dtype=np.float32).astype(ml_dtypes.bfloat16)


def _prep_in_maps(positions, hidden_states, w_fused, w_qb, w_kvb, w_o,
                  qa_ln_w, kva_ln_w):
    positions = np.asarray(positions)
    hidden_states = np.asarray(hidden_states, dtype=np.float32)
    w_fused = np.asarray(w_fused, dtype=np.float32)
    w_qb = np.asarray(w_qb, dtype=np.float32)
    w_kvb = np.asarray(w_kvb, dtype=np.float32)
    w_o = np.asarray(w_o, dtype=np.float32)
    qa_ln_w = np.asarray(qa_ln_w, dtype=np.float32)
    kva_ln_w = np.asarray(kva_ln_w, dtype=np.float32)

    inv_freq = 1.0 / (THETA ** (np.arange(0, DR, 2, dtype=np.float32) / DR))
    freqs = positions.astype(np.float32)[:, None] * inv_freq[None, :]
    cs_full = np.concatenate([np.cos(freqs), np.sin(freqs)], axis=1)  # [T, 64]

    wqb_folded = qa_ln_w[:, None] * w_qb
    # permute q-up output columns: [all nope cols | all pe cols]
    wqb_r = wqb_folded.reshape(QL, H, DN + DR)
    wqb_perm = np.concatenate(
        [wqb_r[:, :, :DN].reshape(QL, H * DN),
         wqb_r[:, :, DN:].reshape(QL, H * DR)], axis=1)
    wkvb_r = w_kvb.reshape(KVL, H, DN + DV)

    # per-partition-contiguous repacks (one fat DMA per block)
    wfkv_bf = _to_bf16(
        w_fused[:, QL:].reshape(8, KD // 8, 128, 576).transpose(0, 2, 1, 3)
        .reshape(8, 128, -1))
    wfq_bf = _to_bf16(np.stack(
        [w_fused[:, g * 512:(g + 1) * 512]
         .reshape(4, KD // 4, 128, 512).transpose(0, 2, 1, 3).reshape(4, 128, -1)
         for g in range(3)]))
    wqb_bf = _to_bf16(np.stack(
        [wqb_perm[:, ng * 512:(ng + 1) * 512]
         .reshape(QKD, 128, 512).transpose(1, 0, 2).reshape(128, -1)
         for ng in range(NGQ)]))
    wo_bf = _to_bf16(np.stack(
        [w_o[:, ht * 512:(ht + 1) * 512]
         .reshape(H, 128, 512).transpose(1, 0, 2).reshape(128, -1)
         for ht in range(NHT)]))

    tri = np.triu(np.ones((128, 128), np.float32))
    cmask = _to_bf16(np.repeat(tri[:, None, :], HLOC, axis=1))

    in_maps = []
    for c in range(NCORES):
        tok = slice(c * TLOC, (c + 1) * TLOC)
        heads = [HLOC * c + i for i in range(HLOC)]
        wkcT = np.stack([(wkvb_r[:, h, :DN] * kva_ln_w[:, None]).T for h in heads])
        wvc = np.concatenate(
            [wkvb_r[:, h, DN:] * kva_ln_w[:, None] for h in heads], axis=1)
        hT_loc = (hidden_states[tok].T.reshape(4, KD // 4, 128, TLOC)
                  .transpose(0, 2, 1, 3).reshape(4, 128, -1))
        in_maps.append({
            "hT": _to_bf16(np.ascontiguousarray(hT_loc)),
            "wfkv": wfkv_bf,
            "wfq": wfq_bf,
            "wqb": wqb_bf,
            "wo": wo_bf,
            "cs": np.ascontiguousarray(cs_full[tok]),
            "wkcT": _to_bf16(np.ascontiguousarray(wkcT)),
            "wvc": _to_bf16(np.ascontiguousarray(wvc)),
            "cmask": cmask,
        })
    return in_maps


def kernel(**inputs):
    global _NC_CACHE, _last_in_maps
    in_maps = _prep_in_maps(**inputs)
    _last_in_maps = in_maps
    if _NC_CACHE is None:
        _NC_CACHE = build_nc()

    res = run_bass_kernel_spmd(_NC_CACHE, in_maps, core_ids=list(range(NCORES)))
    return np.concatenate([np.asarray(res.results[c]["out"], dtype=np.float32)
                           for c in range(NCORES)], axis=0)


if __name__ == "__main__":
    build_nc()
    print("build ok")
